# revision 13
# baseline (speedup 1.0000x reference)
"""Causal self-attention on 8 Trainium2 NeuronCores (Bass/Tile).

Problem: y = CausalSelfAttention(x; Wq, Wk, Wv, Wo) with
B=4, S=2048, E=1024, H=16 heads of 64, fp32 inputs/outputs.

Sharding (tensor-parallel x data-parallel): core c of 8 handles batch
b = c//2 and head-group g = c%2 (8 of 16 heads). Each core receives
x[b] [S, E], the head-group's columns of Wq/Wk/Wv [E, 512] and rows of
Wo [512, E], and produces a partial output projection [S, E]. The host
sums the two partials per batch.

Per-core dataflow (attention-path operands bf16, accumulation fp32).
All matmuls are issued as PE-tile-packed instruction groups that run
concurrently on disjoint 64-row / 64-col groups of the 128x128 array
(HW-measured ~218ns per packed pair at N=512 vs ~270ns for one full
matmul):
  xT = transpose(x) via the 2-byte XBAR DMA transpose (a serial
    ~40us resource: issued first, with every projection's contraction
    loop ordered so it consumes transposed e-tiles as they land)
  weights ride the scalar engine's separate HWDGE queue as 10 batched
    strided DMAs so they never queue behind the transposes
  qT/kT [512, S] = W.T @ x.T and v [S, 512] = x @ Wv, every projection
    matmul col-split into two concurrent M=64 instructions
  per head-pair t, q-chunk of 512, k-tile of 128:
    ST [128, 2x512] = K @ Q.T   (two heads row-packed: disjoint 64-row
                                 groups, concurrent)
    PT = exp(ST/8)              (one wide ACTIVATE per pair; causal mask
                                 = triangular-mask multiply on the
                                 diagonal subblock, both heads in one
                                 strided DVE op)
    AV [128, 512] += [V_h0|V_h1] col-packed PV pair (M=64+64, own
                                 streams); SM [128, 512] += ones-matmul
                                 pair producing broadcast softmax sums
  attT = AV * reciprocal(SM)    (two DVE ops per (pair, q-chunk))
  out = sum_t attT_t.T @ Wo_t accumulated over all 4 pairs in PSUM
    (col-packed), evacuated once per s-tile, DMA'd out.

The scalar engine's EXP stream (~150us total) is the steady-state
bottleneck of the attention phase, so projection work is diced into
16-matmul bursts and interleaved between attention k-tile pairs: the
2-deep ST pipeline never drains and the PE soaks up the spare cycles.

No device collectives; the host slices inputs and sums partials.
"""

import numpy as np

import concourse.bass as bass
import concourse.mybir as mybir
from concourse import bacc
from concourse.tile import TileContext

FP = mybir.dt.float32
BF = mybir.dt.bfloat16
P = 128


def build(S=2048, E=1024, HPC=8, DH=64, NQ=512):
    GD = HPC * DH          # 512 head dims per core
    KT_E = E // P          # 8 contraction tiles over E
    ST_S = S // P          # 16 sequence tiles
    QC = S // NQ           # 4 q-chunks
    DT = GD // P           # 4 head pairs
    QSUB = NQ // P         # 4 k-tiles per q-chunk step

    assert DH == 64 and NQ % P == 0 and S % NQ == 0 and E % P == 0

    nc = bacc.Bacc(None, target_bir_lowering=False)
    x_d = nc.dram_tensor("x", [S, E], BF, kind="ExternalInput")
    wq_d = nc.dram_tensor("wq", [E, GD], BF, kind="ExternalInput")
    wk_d = nc.dram_tensor("wk", [E, GD], BF, kind="ExternalInput")
    wv_d = nc.dram_tensor("wv", [E, GD], BF, kind="ExternalInput")
    wo_d = nc.dram_tensor("wo", [GD, E], BF, kind="ExternalInput")
    out_d = nc.dram_tensor("out", [S, E], FP, kind="ExternalOutput")

    with TileContext(nc) as tc:
        with (
            tc.tile_pool(name="consts", bufs=1) as consts,
            tc.tile_pool(name="data", bufs=1) as data,
            tc.tile_pool(name="xT", bufs=1) as xT_pool,
            tc.tile_pool(name="wbuf", bufs=1) as wbuf,
            tc.tile_pool(name="pt_sb", bufs=8) as pt_pool,
            tc.tile_pool(name="rec_sb", bufs=2) as rec_pool,
            tc.tile_pool(name="po_sb", bufs=2) as posb_pool,
            tc.tile_pool(name="st_psum", bufs=2, space="PSUM") as st_pool,
            tc.tile_pool(name="av_psum", bufs=2, space="PSUM") as av_pool,
            tc.tile_pool(name="sm_psum", bufs=2, space="PSUM") as sm_pool,
        ):
            # ---- constants -------------------------------------------------
            ones64 = consts.tile([P, 64], BF)
            nc.vector.memset(ones64[:], 1.0)
            # upper-triangular-inclusive multiplicative mask (valid k <= q),
            # replicated side by side for the two packed heads
            ut = consts.tile([P, P], BF)
            nc.gpsimd.memset(ut[:], 0.0)
            nc.gpsimd.affine_select(
                out=ut[:], in_=ut[:],
                compare_op=mybir.AluOpType.is_gt, fill=1.0,
                base=0, pattern=[[-1, P]], channel_multiplier=1,
            )
            ut2 = consts.tile([P, 2 * P], BF)
            nc.vector.tensor_copy(ut2[:, 0:P], ut[:])
            nc.vector.tensor_copy(ut2[:, P : 2 * P], ut[:])
            # preload the EXP table set while DMAs are in flight
            dummy_in = consts.tile([1, 1], FP)
            nc.vector.memset(dummy_in[:], 0.0)
            dummy_out = consts.tile([1, 1], BF)
            nc.scalar.activation(
                dummy_out[:], dummy_in[:], mybir.ActivationFunctionType.Exp,
                scale=1.0,
            )

            # ---- persistent SBUF data -------------------------------------
            kT = [data.tile([P, S], BF, tag=f"kT{t}", name=f"kT{t}") for t in range(DT)]
            qT = [data.tile([P, S], BF, tag=f"qT{t}", name=f"qT{t}") for t in range(DT)]
            v = [data.tile([P, GD], BF, tag=f"v{st}", name=f"v{st}") for st in range(ST_S)]
            attT = [data.tile([P, S], BF, tag=f"attT{t}", name=f"attT{t}") for t in range(DT)]
            xTc = [
                [xT_pool.tile([P, NQ], BF, tag=f"xT{et}_{sc}", name=f"xT{et}_{sc}")
                 for sc in range(QC)]
                for et in range(KT_E)
            ]
            wk_all = [wbuf.tile([P, KT_E * P], BF, tag=f"wk{mt}", name=f"wk{mt}") for mt in range(DT)]
            wq_all = [wbuf.tile([P, KT_E * P], BF, tag=f"wq{mt}", name=f"wq{mt}") for mt in range(DT)]
            wv_all = wbuf.tile([P, KT_E * GD], BF, tag="wv", name="wv")
            wo_all = wbuf.tile([P, DT * E], BF, tag="wo", name="wo")

            # transposes first on the sync HWDGE queue: the XBAR engine is
            # serial and gates everything downstream
            for sc in range(QC):
                for et in range(KT_E):
                    nc.sync.dma_start(
                        out=xTc[et][sc][:],
                        in_=x_d[sc * NQ : (sc + 1) * NQ, et * P : (et + 1) * P],
                        transpose=True,
                    )
            # weights as 10 batched strided DMAs on the scalar HWDGE queue,
            # ordered by first use
            wkq_src = [
                (w_d.rearrange("(kt p) c -> p kt c", p=P), all_t)
                for w_d, all_t in ((wk_d, wk_all), (wq_d, wq_all))
            ]
            for mt in range(DT):
                for src, all_t in wkq_src:
                    nc.scalar.dma_start(
                        out=all_t[mt].rearrange("p (kt c) -> p kt c", c=P),
                        in_=src[:, :, mt * P : (mt + 1) * P],
                    )
                if mt == 0:
                    nc.scalar.dma_start(
                        out=wv_all.rearrange("p (kt c) -> p kt c", c=GD),
                        in_=wv_d.rearrange("(kt p) c -> p kt c", p=P),
                    )
                if mt == 1:
                    nc.scalar.dma_start(
                        out=wo_all.rearrange("p (t c) -> p t c", c=E),
                        in_=wo_d.rearrange("(t p) c -> p t c", p=P),
                    )

            def emit_proj_kq(mt, nsc, which):
                """kT[mt] (which=0) or qT[mt] (which=1) for chunk nsc."""
                w_all = (wk_all, wq_all)[which][mt]
                dstT = (kT, qT)[which]
                ps = st_pool.tile([P, 2 * NQ], FP, tag="st", name=f"pj{which}_{mt}_{nsc}")
                for kt in range(KT_E):
                    for ch in range(2):
                        nc.tensor.matmul(
                            ps[64 * ch : 64 * ch + 64, 0:NQ],
                            lhsT=w_all[:, kt * P + 64 * ch : kt * P + 64 * ch + 64],
                            rhs=xTc[kt][nsc][:],
                            start=(kt == 0), stop=(kt == KT_E - 1),
                            skip_group_check=True,
                        )
                nc.vector.tensor_copy(dstT[mt][:, nsc * NQ : (nsc + 1) * NQ], ps[:, 0:NQ])

            def emit_proj_v(st_i):
                """v tile st_i (natural layout [s, gd])."""
                sc, r = divmod(st_i * P, NQ)
                ps = st_pool.tile([P, 2 * NQ], FP, tag="st", name=f"pv{st_i}")
                for kt in range(KT_E):
                    for ch in range(2):
                        nc.tensor.matmul(
                            ps[64 * ch : 64 * ch + 64, 0:NQ],
                            lhsT=xTc[kt][sc][:, r + 64 * ch : r + 64 * ch + 64],
                            rhs=wv_all[:, kt * GD : (kt + 1) * GD],
                            start=(kt == 0), stop=(kt == KT_E - 1),
                            skip_group_check=True,
                        )
                nc.vector.tensor_copy(v[st_i][:], ps[:, 0:NQ])

            def emit_outproj(st):
                po = st_pool.tile([P, 2 * NQ], FP, tag="st", name=f"po{st}")
                for nj in range(2):
                    for t in range(DT):
                        for ch in range(2):
                            nc.tensor.matmul(
                                po[64 * ch : 64 * ch + 64, nj * NQ : (nj + 1) * NQ],
                                lhsT=attT[t][:, st * P + 64 * ch : st * P + 64 * ch + 64],
                                rhs=wo_all[:, t * E + nj * NQ : t * E + (nj + 1) * NQ],
                                start=(t == 0), stop=(t == DT - 1),
                                skip_group_check=True,
                            )
                posb = posb_pool.tile([P, E], FP, tag="posb")
                nc.vector.tensor_copy(posb[:], po[:])
                nc.sync.dma_start(out=out_d[st * P : (st + 1) * P, :], in_=posb[:])

            def attn_unit(t, qj, fillers):
                n_tiles = QSUB * qj + QSUB
                kmax = n_tiles - 1
                av = av_pool.tile([P, NQ], FP, tag="av", name=f"av{t}_{qj}")
                sm = sm_pool.tile([P, NQ], FP, tag="sm", name=f"sm{t}_{qj}")

                def qk(ki):
                    stp = st_pool.tile([P, 2 * NQ], FP, tag="st")
                    d = ki - QSUB * qj
                    off = P * d if d > 0 else 0
                    for half in range(2):
                        pr = 64 * half
                        nc.tensor.matmul(
                            stp[:, half * NQ + off : (half + 1) * NQ],
                            lhsT=kT[t][pr : pr + 64, ki * P : (ki + 1) * P],
                            rhs=qT[t][pr : pr + 64, qj * NQ + off : (qj + 1) * NQ],
                            start=True, stop=True,
                        )
                    return stp, off, d

                def exp_mask(stp, off, d):
                    pt = pt_pool.tile([P, 2 * NQ], BF, tag="pt")
                    if off == 0:
                        nc.scalar.activation(
                            pt[:, 0 : 2 * NQ], stp[:, 0 : 2 * NQ],
                            mybir.ActivationFunctionType.Exp, scale=0.125,
                        )
                    else:
                        # one ACTIVATE over both heads' valid spans via a
                        # strided AP; dead cols are never read downstream
                        pt2 = pt.rearrange("p (k c) -> p k c", c=NQ)
                        st2 = stp.rearrange("p (k c) -> p k c", c=NQ)
                        nc.scalar.activation(
                            pt2[:, :, off:NQ], st2[:, :, off:NQ],
                            mybir.ActivationFunctionType.Exp, scale=0.125,
                        )
                    if d >= 0:
                        # causal mask on the diagonal subblock, both heads in
                        # one strided op
                        pt3 = pt.rearrange("p (k c) -> p k c", c=NQ)
                        ut3 = ut2.rearrange("p (k c) -> p k c", c=P)
                        nc.vector.tensor_tensor(
                            pt3[:, :, off : off + P], pt3[:, :, off : off + P],
                            ut3[:], mybir.AluOpType.mult,
                        )
                    return pt

                def pv_sums(pt, off, ki):
                    st_f, sp_f = (ki == 0), (ki == kmax)
                    for half in range(2):
                        h = 2 * t + half
                        nc.tensor.matmul(
                            av[64 * half : 64 * half + 64, off:NQ],
                            lhsT=v[ki][:, h * DH : h * DH + DH],
                            rhs=pt[:, half * NQ + off : (half + 1) * NQ],
                            start=st_f, stop=sp_f, skip_group_check=True,
                        )
                        nc.tensor.matmul(
                            sm[64 * half : 64 * half + 64, off:NQ],
                            lhsT=ones64[:],
                            rhs=pt[:, half * NQ + off : (half + 1) * NQ],
                            start=st_f, stop=sp_f, skip_group_check=True,
                        )

                # ki-pairs keep same-shape instruction streaks on the PE;
                # one projection burst rides between consecutive pairs
                for kp in range(n_tiles // 2):
                    kis = (2 * kp, 2 * kp + 1)
                    sts = [qk(ki) for ki in kis]
                    pts = [exp_mask(stp, off, d) for stp, off, d in sts]
                    for ki, pt, (stp, off, d) in zip(kis, pts, sts):
                        pv_sums(pt, off, ki)
                    if fillers:
                        fillers.pop(0)()

                rec = rec_pool.tile([P, NQ], FP, tag="rec")
                nc.vector.reciprocal_approx_fast(rec[:], sm[:])
                nc.vector.tensor_tensor(
                    attT[t][:, qj * NQ : (qj + 1) * NQ], av[:], rec[:],
                    mybir.AluOpType.mult,
                )

            # ---- main pipeline --------------------------------------------
            def chunk_group(nsc):
                g = [lambda n=nsc: emit_proj_kq(0, n, 0),
                     lambda n=nsc: emit_proj_kq(0, n, 1)]
                g += [lambda s=st_i: emit_proj_v(s)
                      for st_i in range(4 * nsc, 4 * nsc + 4)]
                return g

            # chunk 0 must fully precede the first attention unit
            for f in chunk_group(0):
                f()

            pending = []  # (needed_before_key, closure), key = (t, qj)
            for nsc in range(1, QC):
                pending += [((0, nsc), f) for f in chunk_group(nsc)]
            for mt in range(1, DT):
                for nsc in range(QC):
                    for which in (0, 1):
                        pending.append((
                            (mt, 0 if mt > 1 else nsc),
                            lambda m=mt, n=nsc, w=which: emit_proj_kq(m, n, w),
                        ))

            for t in range(DT):
                for qj in range(QC):
                    while pending and pending[0][0] <= (t, qj):
                        pending.pop(0)[1]()
                    fillers = []
                    if t < DT - 1:
                        take = min(len(pending), (QSUB * qj + QSUB) >> 1)
                        fillers = [f for _, f in pending[:take]]
                        del pending[:take]
                    else:
                        # last pair: interleave the output projection of the
                        # previous q-chunk's s-tiles
                        if qj > 0:
                            fillers = [
                                lambda s=st: emit_outproj(s)
                                for st in range(4 * (qj - 1), 4 * qj)
                            ]
                    attn_unit(t, qj, fillers)
            for st in range(4 * (QC - 1), 4 * QC):
                emit_outproj(st)

    nc.compile()
    return nc


_NC_CACHE = {}


def _get_nc():
    if "nc" not in _NC_CACHE:
        _NC_CACHE["nc"] = build()
    return _NC_CACHE["nc"]


B, S, E, H, DH = 4, 2048, 1024, 16, 64
GD = (H // 2) * DH  # 512 per-core head dims


def _in_maps(x, Wq, Wk, Wv, Wo):
    import ml_dtypes

    bf = ml_dtypes.bfloat16
    maps = []
    for c in range(8):
        b, g = c // 2, c % 2
        sl = slice(g * GD, (g + 1) * GD)
        maps.append({
            "x": x[b].astype(bf),
            "wq": Wq[:, sl].astype(bf),
            "wk": Wk[:, sl].astype(bf),
            "wv": Wv[:, sl].astype(bf),
            "wo": Wo[sl, :].astype(bf),
        })
    return maps


def kernel(x, Wq, Wk, Wv, Wo):
    from concourse.bass_utils import run_bass_kernel_spmd

    x = np.asarray(x, dtype=np.float32)
    Wq = np.asarray(Wq, dtype=np.float32)
    Wk = np.asarray(Wk, dtype=np.float32)
    Wv = np.asarray(Wv, dtype=np.float32)
    Wo = np.asarray(Wo, dtype=np.float32)

    res = run_bass_kernel_spmd(
        _get_nc(), _in_maps(x, Wq, Wk, Wv, Wo), list(range(8))
    )

    out = np.empty((B, S, E), np.float32)
    for b in range(B):
        out[b] = res.results[2 * b]["out"] + res.results[2 * b + 1]["out"]
    return out
